# revision 11
# baseline (speedup 1.0000x reference)
"""Causal self-attention (B=4, S=2048, E=1024, H=16) on 8 TRN2 NeuronCores.

Sharding: core c owns rows [c*1024, (c+1)*1024) of the flattened (B*S, E)
activation — i.e. batch b = c//2, sequence half c%2. Each core projects
Q/K/V for ALL 16 heads on its own 1024 rows, then an in-kernel pair
AllGather (cores 2b <-> 2b+1 over NeuronLink) shares K^T and V so the
odd core can attend over the full prefix. Attention masking (causal +
padding) is data-driven per core: an additive per-(chunk,ktile) bias
input plus on-chip triangle bands gated by per-core 0/1 scalars, so one
SPMD program serves both sequence halves. The output projection and
b_proj add also run in-kernel; each core emits its own 1024 output rows
in natural layout, fp16.

The axon tunnel to the devices runs at only ~30-40MB/s, so per-call
traffic is minimized: weights live on device (revalidated by
np.array_equal each call), and activations cross the wire int8-quantized
with per-row scales s = f*2^e packed as two extra int8 rows (x is
quantized on host / dequantized in-kernel; the output projection is
quantized in-kernel from f32 psum / dequantized on host). The cached jit
takes the numpy x directly (transfer fused into the dispatch) and the
previous call's output buffer is donated back as the next output.
"""
import numpy as np
from contextlib import ExitStack

import jax

import concourse.bass as bass
import concourse.tile as tile
import concourse.mybir as mybir
from concourse.masks import make_identity

B, S, E, H = 4, 2048, 1024, 16
D = E // H              # 64
NCORES = 8
SO = 1024               # rows owned per core
NST = SO // 128         # 8 own s-tiles
NET = E // 128          # 8 e-tiles
NHP = H // 2            # 8 head pairs
CH = 512                # q chunk
NCHUNK = SO // CH       # 2
SKV = 2 * SO            # gathered key rows
NKT = SKV // 128        # 16 k tiles
NEG = -240000.0         # additive mask (pre-scale); *0.125 = -30000
NEGB = -30000.0         # post-scale bias mask

F32 = mybir.dt.float32
F32R = mybir.dt.float32r
F16 = mybir.dt.float16
BF16 = mybir.dt.bfloat16


def _split_multi_waits(nc, max_waits=1):
    """This walrus build supports at most one sync wait per ISA instruction.
    Hoist extra waits onto same-engine NoOps inserted before the offender."""
    ctr = 0
    n_split = 0
    for f in nc.m.functions:
        for bb in f.blocks:
            insts = list(bb.instructions)
            out = []
            changed = False
            for ins in insts:
                si = getattr(ins, "sync_info", None)
                waits = list(si.on_wait) if (si and si.on_wait) else []
                if len(waits) > max_waits:
                    for w in waits[:-max_waits]:
                        ctr += 1
                        nop = mybir.InstNoOp(
                            name=f"I-wsplit-{ctr}", ins=[], outs=[],
                            engine=ins.engine)
                        nop.sync_info = mybir.SyncInfo(on_wait=[w], on_update=[])
                        out.append(nop)
                        n_split += 1
                    ins.sync_info = mybir.SyncInfo(
                        on_wait=waits[-max_waits:],
                        on_update=list(si.on_update or []))
                    changed = True
                out.append(ins)
            if changed:
                bb.instructions = out
    return n_split


def _build(reps=1):
    nc = bass.Bass(trn_type="TRN2", target_bir_lowering=False, debug=False,
                   num_devices=NCORES)
    # x arrives int8-quantized per row with scale f*2^e; rows SO and SO+1
    # carry the e and f bytes (p-major packed) for this core's rows.
    xh = nc.dram_tensor("xh", [SO + 2, E], mybir.dt.int8,
                        kind="ExternalInput").ap()
    wqk = nc.dram_tensor("wqk", [E, 2 * E], F32R, kind="ExternalInput").ap()
    wv = nc.dram_tensor("wv", [E, E], F32R, kind="ExternalInput").ap()
    wp = nc.dram_tensor("wp", [E, E], F32R, kind="ExternalInput").ap()
    bqk = nc.dram_tensor("bqk", [128, 16], F32, kind="ExternalInput").ap()
    bv = nc.dram_tensor("bv", [1, E], F32R, kind="ExternalInput").ap()
    bp = nc.dram_tensor("bp", [1, E], F32R, kind="ExternalInput").ap()
    mbias = nc.dram_tensor("mbias", [128, NCHUNK * NKT], F32,
                           kind="ExternalInput").ap()
    sab = nc.dram_tensor("sab", [128, 2], F32, kind="ExternalInput").ap()
    # out rows SO/SO+1 carry per-output-row scale s = f * 2^e as int8
    # e/f streams (col r = row r's bytes); host dequantizes with s/127.
    out = nc.dram_tensor("out", [SO + 2, E], mybir.dt.int8,
                         kind="ExternalOutput").ap()

    with tile.TileContext(nc) as tc:
     for _rep in range(reps):
      with ExitStack() as ctx:
        # ---------- pools ----------
        setup = ctx.enter_context(tc.tile_pool(name="setup", bufs=1))
        small_p = ctx.enter_context(tc.tile_pool(name="small", bufs=4))
        bcast_p = ctx.enter_context(tc.tile_pool(name="bcast", bufs=2))
        hb_p = ctx.enter_context(tc.tile_pool(name="hbst", bufs=2))
        dram = ctx.enter_context(tc.tile_pool(name="dram", bufs=1,
                                              space="DRAM"))
        qT_p = ctx.enter_context(tc.tile_pool(name="qT", bufs=1))
        psum_proj = ctx.enter_context(
            tc.tile_pool(name="ps_proj", bufs=2, space="PSUM"))

        # ---------- DRAM bounce buffers for the pair AllGather ----------
        kT_own = dram.tile([NHP * 128, SO], BF16)
        kT_full = dram.tile([2 * NHP * 128, SO], BF16)
        v_own = dram.tile([NST * 128, H * 65], BF16)
        v_full = dram.tile([2 * NST * 128, H * 65], BF16)

        # ---------- setup constants ----------
        identf = setup.tile([128, 128], F32)
        make_identity(nc, identf[:])
        ident = setup.tile([128, 128], F32R)
        nc.vector.tensor_copy(ident[:], identf[:])

        ones_f32 = setup.tile([1, 128], F32)
        nc.gpsimd.memset(ones_f32[:], 1.0)
        ones64 = setup.tile([1, 64], F32R)
        nc.vector.tensor_copy(ones64[:], ones_f32[:, 0:64])
        ones128 = setup.tile([1, 128], F32R)
        nc.vector.tensor_copy(ones128[:], ones_f32[:])

        bqk_sb = setup.tile([128, 16], F32)
        nc.sync.dma_start(out=bqk_sb[:], in_=bqk)
        mbias_sb = setup.tile([128, NCHUNK * NKT], F32)
        nc.sync.dma_start(out=mbias_sb[:], in_=mbias)
        bv_sb = setup.tile([1, E], F32R)
        nc.sync.dma_start(out=bv_sb[:], in_=bv)
        bp_sb = setup.tile([1, E], F32R)
        nc.sync.dma_start(out=bp_sb[:], in_=bp)
        sab_sb = setup.tile([128, 2], F32)
        nc.sync.dma_start(out=sab_sb[:], in_=sab)

        # triangle bands, gated per core: tria for even cores (band A,
        # diag tiles i=4j+t), trib for odd cores (band B, i=8+4j+t).
        # T[p, t*512+cc] = 0 if cc >= 128t + p else NEG.
        tria = setup.tile([128, 4 * CH], F32)
        trib = setup.tile([128, 4 * CH], F32)
        with tc.tile_pool(name="tmpT", bufs=1) as tmp_p:
            T = tmp_p.tile([128, 4 * CH], F32)
            nc.gpsimd.memset(T[:], 0.0)
            for t in range(4):
                nc.gpsimd.affine_select(
                    out=T[:, t * CH:(t + 1) * CH],
                    in_=T[:, t * CH:(t + 1) * CH],
                    compare_op=mybir.AluOpType.is_ge, fill=NEG,
                    base=-128 * t, channel_multiplier=-1, pattern=[[1, CH]])
            nc.vector.tensor_scalar_mul(out=tria[:], in0=T[:],
                                        scalar1=sab_sb[:, 0:1])
            nc.vector.tensor_scalar_mul(out=trib[:], in0=T[:],
                                        scalar1=sab_sb[:, 1:2])

        # decode per-row x scales: xs[p, st] = f * 2^e for row st*128+p
        esb = setup.tile([128, 2, 8], mybir.dt.int8)
        nc.sync.dma_start(
            out=esb[:],
            in_=xh[SO:SO + 2, :].rearrange("r (p s) -> p r s", p=128))
        ef = setup.tile([128, 2, 8], F32)
        nc.vector.tensor_copy(ef[:], esb[:])
        xs = setup.tile([128, NST], F32)
        nc.scalar.activation(out=xs[:], in_=ef[:, 0, :],
                             func=mybir.ActivationFunctionType.Exp,
                             scale=float(np.log(2.0)))
        nc.vector.tensor_mul(xs[:], xs[:], ef[:, 1, :])

        qT = qT_p.tile([128, NHP, SO], BF16)

        with ExitStack() as xts:
            xT_p = xts.enter_context(tc.tile_pool(name="xT", bufs=1))
            xT = xT_p.tile([128, NET, SO], F32R)

            # ---------- phase A: transpose x (f16 -> f32r) ----------
            with ExitStack() as pa:
                xnat_p = pa.enter_context(tc.tile_pool(name="xnat", bufs=2))
                xcv_p = pa.enter_context(tc.tile_pool(name="xcv", bufs=2))
                wv_p = pa.enter_context(tc.tile_pool(name="wv", bufs=1))
                vst_p = pa.enter_context(tc.tile_pool(name="vst", bufs=3))
                psum_tr = pa.enter_context(
                    tc.tile_pool(name="ps_tr", bufs=2, space="PSUM"))

                wvt = wv_p.tile([128, NET, E], F32R)
                nc.sync.dma_start(
                    out=wvt[:], in_=wv.rearrange("(e p) c -> p e c", p=128))

                xr = xh[0:SO, :].rearrange("(s p) e -> p s e", p=128)
                for stg in range(NST // 2):
                    x16 = xnat_p.tile([128, 2, E], mybir.dt.int8, tag="xn",
                                      name="x16")
                    nc.sync.dma_start(out=x16[:],
                                      in_=xr[:, stg * 2:(stg + 1) * 2, :])
                    xt = xcv_p.tile([128, 2, E], F32R, tag="xc", name="xt")
                    for k in range(2):
                        st = stg * 2 + k
                        nc.vector.tensor_scalar_mul(
                            out=xt[:, k, :], in0=x16[:, k, :],
                            scalar1=xs[:, st:st + 1])
                    for e in range(NET):
                        pt = psum_tr.tile([128, 256], F32R, tag="tr")
                        for k in range(2):
                            nc.tensor.matmul(
                                pt[:, k * 128:(k + 1) * 128],
                                xt[:, k, e * 128:(e + 1) * 128],
                                ident[:], is_transpose=True,
                                start=True, stop=True)
                        if e % 2 == 0:
                            nc.vector.tensor_copy(
                                xT[:, e, stg * 256:(stg + 1) * 256], pt[:])
                        else:
                            nc.scalar.copy(
                                xT[:, e, stg * 256:(stg + 1) * 256], pt[:])

                # ---------- phase A2: V = x @ Wv + bv, ones col, stage ----
                for st in range(NST):
                    for vh in range(2):
                        pv = psum_proj.tile([128, 512], F32, tag="pj")
                        for e in range(NET):
                            nc.tensor.matmul(
                                pv[:], xT[:, e, st * 128:(st + 1) * 128],
                                wvt[:, e, vh * 512:(vh + 1) * 512],
                                start=(e == 0), stop=False)
                        nc.tensor.matmul(pv[:], ones128[:],
                                         bv_sb[:, vh * 512:(vh + 1) * 512],
                                         start=False, stop=True)
                        vstage = vst_p.tile([128, 8, 65], BF16, tag="vs",
                                            name="vstage")
                        nc.gpsimd.memset(vstage[:, :, 64:65], 1.0)
                        nc.scalar.copy(
                            vstage[:, :, 0:64],
                            pv[:].rearrange("p (h d) -> p h d", h=8))
                        nc.sync.dma_start(
                            out=v_own[st * 128:(st + 1) * 128,
                                      vh * 520:(vh + 1) * 520],
                            in_=vstage[:].rearrange("p h d -> p (h d)"))

            # V gather can overlap phase B (no data deps)
            nc.gpsimd.collective_compute(
                "AllGather", mybir.AluOpType.bypass,
                replica_groups=[[0, 1], [2, 3], [4, 5], [6, 7]],
                ins=[v_own[:]], outs=[v_full[:]])

            # ---------- phase B: Q^T, K^T for all head pairs ----------
            with ExitStack() as pb_:
                wqk_p = pb_.enter_context(tc.tile_pool(name="wqks", bufs=3))
                kst_p = pb_.enter_context(tc.tile_pool(name="kst", bufs=3))
                wqkr = wqk.rearrange("(e q) c -> q e c", q=128)
                for sec in range(2):
                    for hp in range(NHP):
                        wt = wqk_p.tile([128, NET, 128], F32R, tag="wqk",
                                        name="wt")
                        c0 = sec * E + hp * 128
                        nc.sync.dma_start(out=wt[:],
                                          in_=wqkr[:, :, c0:c0 + 128])
                        for jj in range(NCHUNK):
                            pq = psum_proj.tile([128, CH], F32, tag="pj")
                            for e in range(NET):
                                nc.tensor.matmul(
                                    pq[:], wt[:, e, :],
                                    xT[:, e, jj * CH:(jj + 1) * CH],
                                    start=(e == 0), stop=(e == NET - 1))
                            bcol = sec * 8 + hp
                            if sec == 0:
                                nc.vector.tensor_scalar_add(
                                    out=qT[:, hp, jj * CH:(jj + 1) * CH],
                                    in0=pq[:],
                                    scalar1=bqk_sb[:, bcol:bcol + 1])
                            else:
                                kst = kst_p.tile([128, CH], BF16, tag="ks",
                                                 name="kst")
                                nc.vector.tensor_scalar_add(
                                    out=kst[:], in0=pq[:],
                                    scalar1=bqk_sb[:, bcol:bcol + 1])
                                nc.sync.dma_start(
                                    out=kT_own[hp * 128:(hp + 1) * 128,
                                               jj * CH:(jj + 1) * CH],
                                    in_=kst[:])

        nc.gpsimd.collective_compute(
            "AllGather", mybir.AluOpType.bypass,
            replica_groups=[[0, 1], [2, 3], [4, 5], [6, 7]],
            ins=[kT_own[:]], outs=[kT_full[:]])

        # ---------- phase C: load gathered K^T and V ----------
        # kT/vaug pools open only now, reusing the space freed by xT and
        # the phase A/B pools.
        kT_p = ctx.enter_context(tc.tile_pool(name="kT", bufs=1))
        vaug_p = ctx.enter_context(tc.tile_pool(name="vaug", bufs=1))
        kT = kT_p.tile([128, NHP, 2, SO], BF16)
        vaug = vaug_p.tile([128, NKT, H, 65], BF16)
        for r in range(2):
            nc.sync.dma_start(
                out=kT[:, :, r, :],
                in_=kT_full[r * NHP * 128:(r + 1) * NHP * 128, :]
                .rearrange("(hp p) c -> p hp c", p=128))
        vfr = v_full[:].rearrange("(g p) (h d) -> p g h d", p=128, h=H)
        for g in range(NKT):
            nc.sync.dma_start(out=vaug[:, g, :, :], in_=vfr[:, g, :, :])

        # ---------- phase D/E: attention + output projection ----------
        with ExitStack() as pp:
            outacc_p = pp.enter_context(tc.tile_pool(name="outacc", bufs=2))
            attn_p = pp.enter_context(tc.tile_pool(name="attnT", bufs=4))
            wp_p = pp.enter_context(tc.tile_pool(name="wp", bufs=1))
            ostage_p = pp.enter_context(tc.tile_pool(name="ostage", bufs=3))
            psum_S = pp.enter_context(
                tc.tile_pool(name="ps_S", bufs=3, space="PSUM"))
            psum_av = pp.enter_context(
                tc.tile_pool(name="ps_av", bufs=2, space="PSUM"))
            psum_b = pp.enter_context(
                tc.tile_pool(name="ps_b", bufs=1, space="PSUM"))

            wpt = wp_p.tile([128, NHP, E], F32R)
            nc.sync.dma_start(
                out=wpt[:], in_=wp.rearrange("(p r) c -> r p c", r=128))

            for j in range(NCHUNK):
                outacc = outacc_p.tile([128, NHP, CH], F32R, tag="oa",
                                       name="outacc")
                nkt = 12 + 4 * j
                for hp in range(NHP):
                    pav = {}
                    for hh in range(2):
                        pav[hh] = psum_av.tile([65, CH], F32, tag="av",
                                               name="pav")
                    for i in range(nkt):
                        for hh in range(2):
                            lo, hi = (0, 64) if hh == 0 else (64, 128)
                            ps = psum_S.tile([128, CH], F32, tag="S")
                            nc.tensor.matmul(
                                ps[:],
                                kT[lo:hi, hp, i // 8, (i % 8) * 128:
                                   (i % 8) * 128 + 128],
                                qT[lo:hi, hp, j * CH:(j + 1) * CH],
                                start=True, stop=True)
                            ta = i - 4 * j
                            tb = i - 8 - 4 * j
                            if 0 <= ta < 4:
                                nc.vector.tensor_add(
                                    ps[:], ps[:],
                                    tria[:, ta * CH:(ta + 1) * CH])
                            if 0 <= tb < 4:
                                nc.vector.tensor_add(
                                    ps[:], ps[:],
                                    trib[:, tb * CH:(tb + 1) * CH])
                            at = attn_p.tile([128, CH], BF16, tag="at")
                            mcol = j * NKT + i
                            nc.scalar.activation(
                                out=at[:], in_=ps[:],
                                func=mybir.ActivationFunctionType.Exp,
                                bias=mbias_sb[:, mcol:mcol + 1], scale=0.125)
                            nc.tensor.matmul(
                                pav[hh][:],
                                vaug[:, i, 2 * hp + hh, 0:65], at[:],
                                start=(i == 0), stop=(i == nkt - 1))
                    for hh in range(2):
                        rec = small_p.tile([1, CH], F32R, tag="rec")
                        with nc.allow_low_precision(
                                reason="softmax recip to f32r"):
                            nc.vector.reciprocal(rec[:], pav[hh][64:65, :])
                        pb = psum_b.tile([64, CH], F32, tag="bc")
                        nc.tensor.matmul(pb[:], ones64[:], rec[:],
                                         start=True, stop=True)
                        bc = bcast_p.tile([64, CH], F32R, tag="bc2")
                        nc.vector.tensor_copy(bc[:], pb[:])
                        if hh == 0:
                            nc.vector.tensor_mul(
                                outacc[0:64, hp, :], pav[hh][0:64, :], bc[:])
                        else:
                            hb = hb_p.tile([64, CH], F32R, tag="hb")
                            nc.vector.tensor_mul(hb[:], pav[hh][0:64, :],
                                                 bc[:])
                            nc.sync.dma_start(
                                out=outacc[64:128, hp, :], in_=hb[:])

                # output projection for this chunk (overlaps next chunk);
                # rows are quantized to int8 with a per-row scale so the
                # host download is 1 byte/element.
                for qt in range(4):
                    po = {}
                    for eh in range(2):
                        po[eh] = psum_proj.tile([128, 512], F32, tag="pj", name="po")
                        for hp in range(NHP):
                            nc.tensor.matmul(
                                po[eh][:],
                                outacc[:, hp, qt * 128:(qt + 1) * 128],
                                wpt[:, hp, eh * 512:(eh + 1) * 512],
                                start=(hp == 0), stop=False)
                        nc.tensor.matmul(po[eh][:], ones128[:],
                                         bp_sb[:, eh * 512:(eh + 1) * 512],
                                         start=False, stop=True)
                    # per-row scale s = f*2^e with f in [64,127] and
                    # s >= rowmax/127, derived branchlessly via Ln/Exp;
                    # floor(x) = rint(x - 0.5), ceil(x) = rint(x + 0.5).
                    m0 = small_p.tile([128, 1], F32, tag="m0")
                    m1 = small_p.tile([128, 1], F32, tag="m1")
                    nc.vector.reduce_max(m0[:], po[0][:],
                                         axis=mybir.AxisListType.X,
                                         apply_absolute_value=True)
                    nc.vector.reduce_max(m1[:], po[1][:],
                                         axis=mybir.AxisListType.X,
                                         apply_absolute_value=True)
                    nc.vector.tensor_max(m0[:], m0[:], m1[:])
                    LN2 = float(np.log(2.0))
                    ef = small_p.tile([128, 1], F32, tag="ef")
                    nc.scalar.activation(
                        out=ef[:], in_=m0[:],
                        func=mybir.ActivationFunctionType.Ln)
                    # e = floor(log2(m/127) - 5.9887) so m*2^-e in
                    # [63.5, 127.02) and f = ceil(.) fits int8
                    nc.vector.tensor_scalar(
                        out=ef[:], in0=ef[:], scalar1=1.0 / LN2,
                        scalar2=-np.log2(127.0) - 5.9887 - 0.5,
                        op0=mybir.AluOpType.mult, op1=mybir.AluOpType.add)
                    e_i8 = small_p.tile([128, 1], mybir.dt.int8, tag="ei")
                    nc.vector.tensor_copy(e_i8[:], ef[:])
                    e_f = small_p.tile([128, 1], F32, tag="efl")
                    nc.vector.tensor_copy(e_f[:], e_i8[:])
                    p2m = small_p.tile([128, 1], F32, tag="p2m")
                    nc.scalar.activation(
                        out=p2m[:], in_=e_f[:],
                        func=mybir.ActivationFunctionType.Exp, scale=-LN2)
                    w = small_p.tile([128, 1], F32, tag="w")
                    nc.vector.tensor_mul(w[:], m0[:], p2m[:])
                    nc.vector.tensor_scalar(
                        out=w[:], in0=w[:], scalar1=1.0 / 127.0,
                        scalar2=0.4999,
                        op0=mybir.AluOpType.mult, op1=mybir.AluOpType.add)
                    f_i8 = small_p.tile([128, 1], mybir.dt.int8, tag="fi")
                    nc.vector.tensor_copy(f_i8[:], w[:])
                    f_f = small_p.tile([128, 1], F32, tag="ffl")
                    nc.vector.tensor_copy(f_f[:], f_i8[:])
                    p2 = small_p.tile([128, 1], F32, tag="p2")
                    nc.scalar.activation(
                        out=p2[:], in_=e_f[:],
                        func=mybir.ActivationFunctionType.Exp, scale=LN2)
                    qs = small_p.tile([128, 1], F32, tag="qs")
                    nc.vector.tensor_mul(qs[:], f_f[:], p2[:])
                    with nc.allow_low_precision(reason="int8 quant scale"):
                        nc.vector.reciprocal(qs[:], qs[:])
                    r0 = j * CH + qt * 128
                    for eh in range(2):
                        os = ostage_p.tile([128, 512], mybir.dt.int8,
                                           tag="os")
                        nc.scalar.activation(
                            out=os[:], in_=po[eh][:],
                            func=mybir.ActivationFunctionType.Copy,
                            scale=qs[:, 0:1])
                        nc.sync.dma_start(
                            out=out[r0:r0 + 128, eh * 512:(eh + 1) * 512],
                            in_=os[:])
                    nc.sync.dma_start(
                        out=out[SO:SO + 1, r0:r0 + 128]
                        .rearrange("o c -> c o"), in_=e_i8[:])
                    nc.sync.dma_start(
                        out=out[SO + 1:SO + 2, r0:r0 + 128]
                        .rearrange("o c -> c o"), in_=f_i8[:])

    _split_multi_waits(nc)
    return nc


# ---------------------------------------------------------------------------
# Host runner: cached jit over 8 axon devices, weights resident on device.
# ---------------------------------------------------------------------------

_ST = {}


def _host_prep_weights(attention_mask, W_qkv, b_qkv, W_proj, b_proj):
    """Per-core weight/mask arrays, keyed as the kernel's dram tensors."""
    per_core = []
    for c in range(NCORES):
        b = c // 2
        parity = c % 2
        mb = np.zeros((128, NCHUNK * NKT), np.float32)
        am = np.asarray(attention_mask[b])
        for j in range(NCHUNK):
            qmax = parity * SO + j * CH + (CH - 1)
            for i in range(NKT):
                col = j * NKT + i
                v = np.where(am[i * 128:(i + 1) * 128] != 0, 0.0, NEGB)
                if 128 * i > qmax:
                    v = v + NEGB
                mb[:, col] = v
        sab = np.zeros((128, 2), np.float32)
        sab[:, 0] = 1.0 if parity == 0 else 0.0
        sab[:, 1] = 1.0 if parity == 1 else 0.0
        per_core.append({"mbias": mb, "sab": sab})

    wqk = np.ascontiguousarray(W_qkv[:, :2 * E])
    wv = np.ascontiguousarray(W_qkv[:, 2 * E:])
    bqk = np.ascontiguousarray(
        b_qkv[:2 * E].reshape(16, 128).T)        # [128, 16] col-tiles
    bv = b_qkv[2 * E:].reshape(1, E)
    bp = np.asarray(b_proj).reshape(1, E)
    shared = {"wqk": wqk, "wv": wv, "wp": np.ascontiguousarray(W_proj),
              "bqk": bqk, "bv": bv, "bp": bp}
    return shared, per_core


def _init_state():
    from jax.sharding import Mesh, PartitionSpec, NamedSharding
    import warnings
    with warnings.catch_warnings():
        warnings.simplefilter("ignore")
        from jax.experimental.shard_map import shard_map as _shard_map
    from concourse.bass2jax import (install_neuronx_cc_hook, _bass_exec_p,
                                    partition_id_tensor)

    install_neuronx_cc_hook()
    nc = _build()

    partition_name = (nc.partition_id_tensor.name
                      if nc.partition_id_tensor else None)
    in_names, out_names, out_avals = [], [], []
    for alloc in nc.m.functions[0].allocations:
        if not isinstance(alloc, mybir.MemoryLocationSet):
            continue
        name = alloc.memorylocations[0].name
        if alloc.kind == "ExternalInput":
            if name != partition_name:
                in_names.append(name)
        elif alloc.kind == "ExternalOutput":
            out_names.append(name)
            shape = tuple(alloc.tensor_shape)
            dtype = mybir.dt.np(alloc.dtype)
            out_avals.append(jax.core.ShapedArray(shape, dtype))
    n_params = len(in_names)
    in_names_full = list(in_names) + out_names
    if partition_name is not None:
        in_names_full.append(partition_name)

    def _body(*args):
        operands = list(args)
        if partition_name is not None:
            operands.append(partition_id_tensor())
        outs = _bass_exec_p.bind(
            *operands, out_avals=tuple(out_avals),
            in_names=tuple(in_names_full), out_names=tuple(out_names),
            lowering_input_output_aliases=(), sim_require_finite=True,
            sim_require_nnan=True, nc=nc)
        return tuple(outs)

    devices = jax.devices()[:NCORES]
    assert len(devices) == NCORES, f"need {NCORES} devices"
    mesh = Mesh(np.asarray(devices), ("core",))
    spec = PartitionSpec("core")
    n_outs = len(out_names)
    donate = tuple(range(n_params, n_params + n_outs))
    sharding = NamedSharding(mesh, spec)
    sharded = jax.jit(
        _shard_map(_body, mesh=mesh, in_specs=(spec,) * (n_params + n_outs),
                   out_specs=(spec,) * n_outs, check_rep=False),
        donate_argnums=donate, keep_unused=True,
        in_shardings=(sharding,) * (n_params + n_outs))

    from concurrent.futures import ThreadPoolExecutor
    _ST.update(
        nc=nc, mesh=mesh, sharding=sharding,
        sharded=sharded, in_names=in_names, out_names=out_names,
        out_avals=out_avals, weights=None, wkey=None, donation=None,
        pool=ThreadPoolExecutor(max_workers=8),
        xh_buf=np.empty((NCORES, SO + 2, E), np.int8),
        res_buf=np.empty((NCORES, SO, E), np.float32))


def _ensure_weights(attention_mask, W_qkv, b_qkv, W_proj, b_proj):
    key = (np.asarray(attention_mask), np.asarray(W_qkv, np.float32),
           np.asarray(b_qkv, np.float32), np.asarray(W_proj, np.float32),
           np.asarray(b_proj, np.float32))
    old = _ST["wkey"]
    if old is not None and all(
            np.array_equal(a, b) for a, b in zip(old, key)):
        return
    shared, per_core = _host_prep_weights(key[0], key[1], key[2], key[3],
                                          key[4])
    devs = {}
    for name in _ST["in_names"]:
        if name == "xh":
            continue
        if name in shared:
            w = shared[name]
            g = np.broadcast_to(
                w, (NCORES,) + w.shape).reshape((NCORES * w.shape[0],)
                                                + w.shape[1:])
        else:
            g = np.concatenate([pc[name] for pc in per_core], axis=0)
        devs[name] = jax.device_put(np.ascontiguousarray(g),
                                    _ST["sharding"])
    _ST["weights"] = devs
    _ST["wkey"] = key


def kernel(x, attention_mask, W_qkv, b_qkv, W_proj, b_proj):
    if not _ST:
        _init_state()
    xf = np.asarray(x, np.float32).reshape(NCORES * SO, E)
    # per-row scale s = f * 2^e (f in [64,127], e int) with s >= rowmax/127,
    # so round(x/s) never clips; the kernel reconstructs s exactly-enough
    # from the two packed int8 rows.
    xh = _ST["xh_buf"]

    def _quant_core(c):
        blk = xf[c * SO:(c + 1) * SO]
        buf = _ST["res_buf"][c]
        m = np.maximum(np.abs(blk).max(axis=1), 1e-30)
        t = m / 127.0
        e = np.floor(np.log2(t)).astype(np.int32) - 6
        f = np.ceil(t * np.exp2(-e)).astype(np.int32)
        bump = f >= 128
        f = np.where(bump, 64, f)
        e = np.where(bump, e + 1, e)
        s = f.astype(np.float32) * np.exp2(e.astype(np.float32))
        np.multiply(blk, (1.0 / s)[:, None], out=buf)
        np.rint(buf, out=buf)
        xh[c, :SO, :] = buf
        xh[c, SO, :] = e.astype(np.int8).reshape(NST, 128).T.reshape(E)
        xh[c, SO + 1, :] = f.astype(np.int8).reshape(NST, 128).T.reshape(E)

    # weights-equality check runs on this thread while the pool quantizes
    # (disjoint arrays; device uploads on a weight change also overlap)
    futs = [_ST["pool"].submit(_quant_core, c) for c in range(NCORES)]
    _ensure_weights(attention_mask, W_qkv, b_qkv, W_proj, b_proj)
    for fu in futs:
        fu.result()
    x_dev = xh.reshape(NCORES * (SO + 2), E)

    if _ST["donation"] is None:
        _ST["donation"] = [
            jax.device_put(
                np.zeros((NCORES * av.shape[0],) + av.shape[1:], av.dtype),
                _ST["sharding"])
            for av in _ST["out_avals"]]

    args = []
    for name in _ST["in_names"]:
        args.append(x_dev if name == "xh" else _ST["weights"][name])
    args.extend(_ST["donation"])

    out_devs = _ST["sharded"](*args)
    res = np.empty((NCORES, SO, E), np.float32)
    shards = out_devs[0].addressable_shards

    def _fetch_dequant(sh):
        c = sh.index[0].start // (SO + 2)
        raw = np.asarray(sh.data)
        e = raw[SO, :].astype(np.float32)
        f = raw[SO + 1, :].astype(np.float32)
        scale = f * np.exp2(e)
        np.multiply(raw[:SO, :], scale[:, None], out=res[c])

    list(_ST["pool"].map(_fetch_dequant, shards))
    _ST["donation"] = list(out_devs)
    return res.reshape(B, S, E)
